# revision 11
# baseline (speedup 1.0000x reference)
"""Cross-attention Trainium2 kernel (8-core SPMD, no collectives).

Sharding: B*Lq = 4*2048 = 8192 query rows split 8 ways -> 1024 rows/core
(core c owns batch c//2, row-half c%2). Weights replicated. Each core:
  kp/vp projections of its batch's kv, qp projection of its q rows,
  per-head attention (softmax without max-subtraction; scores are O(1)
  by construction), output projection with the bias added during the
  PSUM->SBUF copy on the vector engine.

All matmul operands are fp16 (same 1 cycle/row PE rate as fp32r but
half the HBM/SBUF footprint); PSUM accumulation is fp32. Inputs are
converted to fp16 host-side. All tiles live at top level (no pool
scoping) so consecutive For_i iterations pipeline: the next
iteration's input DMAs (SP queue) overlap this iteration's compute,
while output DMAs drain on the separate ACT queue.
"""

import sys

import numpy as np

for _p in ("/opt/trn_rl_repo",):
    if _p not in sys.path:
        sys.path.insert(0, _p)

B, LQ, LKV = 4, 2048, 512
DQ, DKV = 1024, 768
H, HD = 8, 128
N_CORES = 8
ROWS = (B * LQ) // N_CORES  # 1024 q rows per core
P = 128
KTQ = DQ // P    # 8 k-tiles for dim_q contraction
KTK = DKV // P   # 6 k-tiles for dim_kv contraction
MC = ROWS // 512  # 2 m-chunks of 512 query rows
NT = LKV // P    # 4 kv n-tiles
SCALE = float(HD) ** -0.5

_STATE = {}


def _build(loop_r=None, phases="abc"):
    import concourse.bass as bass
    import concourse.mybir as mybir
    import concourse.tile as tile
    from concourse import bacc
    from concourse.bass import ts
    from contextlib import ExitStack

    f32 = mybir.dt.float32
    f16 = mybir.dt.float16
    Exp = mybir.ActivationFunctionType.Exp

    nc = bacc.Bacc("TRN2", target_bir_lowering=False, debug=False)
    qT_ap = nc.dram_tensor("qT", [DQ, ROWS], f16, kind="ExternalInput").ap()
    kvT_ap = nc.dram_tensor("kvT", [DKV, LKV], f16, kind="ExternalInput").ap()
    wq_ap = nc.dram_tensor("wq", [DQ, DQ], f16, kind="ExternalInput").ap()
    wk_ap = nc.dram_tensor("wk", [DKV, DQ], f16, kind="ExternalInput").ap()
    wv_ap = nc.dram_tensor("wv", [DKV, DQ], f16, kind="ExternalInput").ap()
    wo_ap = nc.dram_tensor("wo", [DQ, DQ], f16, kind="ExternalInput").ap()
    bo_ap = nc.dram_tensor("bo_bc", [P, DQ], f32, kind="ExternalInput").ap()
    out_ap = nc.dram_tensor("out", [ROWS, DQ], f32, kind="ExternalOutput").ap()

    qT_r = qT_ap.rearrange("(kt p) m -> p kt m", p=P)     # [128, 8, 1024]
    kvT_r = kvT_ap.rearrange("(kt p) n -> p kt n", p=P)   # [128, 6, 512]
    wq_r = wq_ap.rearrange("(kt p) n -> p kt n", p=P)     # [128, 8, 1024]
    wk_r = wk_ap.rearrange("(kt p) n -> p kt n", p=P)     # [128, 6, 1024]
    wv_r = wv_ap.rearrange("(kt p) n -> p kt n", p=P)     # [128, 6, 1024]
    wo_r = wo_ap.rearrange("(kt p) n -> p kt n", p=P)     # [128, 8, 1024]

    def _emit_body(tc, pl):
        const, dat, psS, psB, psO, pAO, pE, pS, pOS, pQ = pl

        ones_col = const.tile([P, 1], f16, tag="ones_col")
        ones_row = const.tile([1, P], f16, tag="ones_row")
        bias2_sb = const.tile([P, 2, 512], f32, tag="bias")

        kvT_sb = dat.tile([P, KTK, LKV], f16, tag="kvT")
        wk_sb = dat.tile([P, KTK, DQ], f16, tag="wk")
        wv_sb = dat.tile([P, KTK, DQ], f16, tag="wv")
        wq_sb = dat.tile([P, KTQ, DQ], f16, tag="wq")
        qt_sb = dat.tile([P, KTQ, ROWS], f16, tag="qt")
        wo_sb = dat.tile([P, KTQ, DQ], f16, tag="wo")
        kpT = dat.tile([P, H, LKV], f16, tag="kpT")      # [d-in-head, h, kv]
        vp = dat.tile([P, NT, DQ], f16, tag="vp")        # [kv-in-tile, t, d]
        qpT0 = dat.tile([P, H, 512], f16, tag="qpT0")    # [d-in-head, h, m]
        qpT1 = dat.tile([P, H, 512], f16, tag="qpT1")
        qpTs = [qpT0, qpT1]

        # ones constants (memset writes f32; convert to f16 via DVE)
        ones_f = pS.tile([P, 1], f32, tag="onesf")
        nc.vector.memset(ones_f, 1.0)
        nc.vector.tensor_copy(ones_col, ones_f)
        onesrow_f = pS.tile([1, P], f32, tag="onesrf")
        nc.vector.memset(onesrow_f, 1.0)
        nc.vector.tensor_copy(ones_row, onesrow_f)

        # ---- Input DMAs (SP HWDGE queue), critical-path first:
        # interleave wk/kvT per-kt so the first kp matmul unblocks early.
        for kt in range(KTK):
            nc.sync.dma_start(out=wk_sb[:, kt, :], in_=wk_r[:, kt, :])
            nc.sync.dma_start(out=kvT_sb[:, kt, :], in_=kvT_r[:, kt, :])
        nc.sync.dma_start(out=wv_sb[:, 0:3, :], in_=wv_r[:, 0:3, :])
        nc.sync.dma_start(out=wv_sb[:, 3:6, :], in_=wv_r[:, 3:6, :])
        nc.sync.dma_start(out=wq_sb[:, 0:4, :], in_=wq_r[:, 0:4, :])
        nc.sync.dma_start(out=wq_sb[:, 4:8, :], in_=wq_r[:, 4:8, :])
        for mch in range(MC):
            nc.sync.dma_start(out=qt_sb[:, :, ts(mch, 512)],
                              in_=qT_r[:, :, ts(mch, 512)])
        nc.sync.dma_start(out=bias2_sb, in_=bo_ap)
        nc.sync.dma_start(out=wo_sb, in_=wo_r)

        # ---- Phase A: k/v projections
        for h in range(H):
            ps = psS.tile([P, 512], f32, tag="s")
            for kt in range(KTK):
                nc.tensor.matmul(ps, wk_sb[:, kt, ts(h, HD)],
                                 kvT_sb[:, kt, :],
                                 start=(kt == 0), stop=(kt == KTK - 1))
            nc.vector.tensor_copy(kpT[:, h, :], ps)
        for t in range(NT):
            for dc in range(2):
                ps = psS.tile([P, 512], f32, tag="s")
                for kt in range(KTK):
                    nc.tensor.matmul(ps, kvT_sb[:, kt, ts(t, P)],
                                     wv_sb[:, kt, ts(dc, 512)],
                                     start=(kt == 0), stop=(kt == KTK - 1))
                nc.vector.tensor_copy(vp[:, t, ts(dc, 512)], ps)

        # ---- Phase B: q projection -> qpT (N=512 matmuls)
        for mch in (range(MC) if "b" in phases else []):
            for h in range(H):
                ps = psS.tile([P, 512], f32, tag="s")
                for kt in range(KTQ):
                    nc.tensor.matmul(ps, wq_sb[:, kt, ts(h, HD)],
                                     qt_sb[:, kt, ts(mch, 512)],
                                     start=(kt == 0), stop=(kt == KTQ - 1))
                nc.vector.tensor_copy(qpTs[mch][:, h, :], ps)

        if "e" in phases:
            # probe: scores+exp pipeline only (no av/dn/out-proj).
            # Each exp is consumed by a DVE accumulate so nothing is DCE'd.
            acc0 = pS.tile([P, 512], f16, tag="eacc0", name="acc0")
            acc1 = pS.tile([P, 512], f16, tag="eacc1", name="acc1")
            acc = [acc0, acc1]
            nc.vector.memset(acc[0].bitcast(mybir.dt.uint16), 0)
            nc.vector.memset(acc[1].bitcast(mybir.dt.uint16), 0)
            i = 0
            for mc in range(MC):
                for h in range(H):
                    for t in range(NT):
                        ps_s = psS.tile([P, 512], f32, tag="s")
                        nc.tensor.matmul(ps_s, kpT[:, h, ts(t, P)],
                                         qpTs[mc][:, h, :],
                                         start=True, stop=True)
                        expT = pE.tile([P, 512], f16, tag="exp")
                        nc.scalar.activation(expT, ps_s, Exp, scale=SCALE)
                        nc.vector.tensor_add(acc[i % 2], acc[(i + 1) % 2], expT)
                        i += 1
            nc.scalar.dma_start(out=out_ap[4 * P:5 * P, 0:256].bitcast(f16),
                                in_=acc[(i + 1) % 2])
            return

        if "c" not in phases:
            # keep phase outputs live (avoid DCE) via sampled stores
            nc.scalar.dma_start(out=out_ap[0:P, 0:LKV // 2].bitcast(f16),
                                in_=kpT[:, 0, :])
            nc.scalar.dma_start(out=out_ap[P:2 * P, 0:DQ // 2].bitcast(f16),
                                in_=vp[:, 0, :])
            if "b" in phases:
                nc.scalar.dma_start(out=out_ap[2 * P:3 * P, 0:256].bitcast(f16),
                                    in_=qpT0[:, 0, :])
                nc.scalar.dma_start(out=out_ap[3 * P:4 * P, 0:256].bitcast(f16),
                                    in_=qpT1[:, 0, :])
            return

        # ---- Phase C: attention + output projection.
        # Scores are computed in pairs into 2-bank PSUM tiles with a single
        # [128,1024] exp per pair; av matmuls run as a contiguous 4-matmul
        # accumulation burst (no interleaved groups); the softmax denominator
        # is a single matmul into row 0 of the broadcast PSUM tile.
        def emit_tail(pending, aoT_of):
            ph, po, slot, prr = pending
            nc.tensor.matmul(slot, ones_row, prr, start=True, stop=True)
            bcast = pS.tile([P, 512], f32, tag="bc")
            nc.vector.tensor_copy(bcast, slot)
            nc.vector.tensor_mul(aoT_of[:, ph, :], po, bcast)

        for mc in range(MC):
            aoT = pAO.tile([P, H, 512], f16, tag="ao")
            pending = None  # (h, ps_o, psB slot, recip) tail of previous head
            for h in range(H):
                ps_o = psO.tile([P, 512], f32, tag="o")
                pairs = []
                for half in range(2):
                    ps_pair = psS.tile([P, 2, 512], f32, tag="s")
                    for j in range(2):
                        t = half * 2 + j
                        nc.tensor.matmul(ps_pair[:, j, :], kpT[:, h, ts(t, P)],
                                         qpTs[mc][:, h, :],
                                         start=True, stop=True)
                    expP = pE.tile([P, 2, 512], f16, tag="exp")
                    nc.scalar.activation(expP, ps_pair, Exp, scale=SCALE)
                    pairs.append(expP)
                    if half == 0 and pending is not None:
                        # previous head's tail: recip is ready by now, so
                        # the bcast matmul doesn't stall PE
                        emit_tail(pending, aoT)
                        pending = None
                for t in range(NT):
                    nc.tensor.matmul(ps_o, vp[:, t, ts(h, HD)],
                                     pairs[t // 2][:, t % 2, :],
                                     start=(t == 0), stop=(t == NT - 1))
                # denominator: quadsum on DVE, one matmul into slot row 0
                s01 = pQ.tile([P, 512], f16, tag="s01")
                nc.vector.tensor_add(s01, pairs[0][:, 0, :], pairs[0][:, 1, :])
                s23 = pQ.tile([P, 512], f16, tag="s23")
                nc.vector.tensor_add(s23, pairs[1][:, 0, :], pairs[1][:, 1, :])
                ssum = pQ.tile([P, 512], f16, tag="ssum")
                nc.vector.tensor_add(ssum, s01, s23)
                slot = psB.tile([P, 512], f32, tag="b")
                nc.tensor.matmul(slot[0:1, :], ones_col, ssum,
                                 start=True, stop=True)
                recip_r = pS.tile([1, 512], f16, tag="rcr")
                with nc.allow_low_precision(reason="fp16 recip"):
                    nc.vector.reciprocal(recip_r, slot[0:1, :])
                pending = (h, ps_o, slot, recip_r)
            emit_tail(pending, aoT)
            # output projection for this m-chunk: two 512-col halves per
            # 2-bank psS slot, one wide bias-add + one [128,1024] out DMA
            for mt in range(4):
                ps_pair = psS.tile([P, 2, 512], f32, tag="s")
                for nci in range(2):
                    for kt in range(KTQ):
                        nc.tensor.matmul(ps_pair[:, nci, :],
                                         aoT[:, kt, ts(mt, P)],
                                         wo_sb[:, kt, ts(nci, 512)],
                                         start=(kt == 0), stop=(kt == KTQ - 1))
                out_sb = pOS.tile([P, 2, 512], f32, tag="osb")
                nc.vector.tensor_add(out_sb, ps_pair, bias2_sb)
                nc.scalar.dma_start(
                    out=out_ap[mc * 512 + mt * P: mc * 512 + (mt + 1) * P, :],
                    in_=out_sb)

    with tile.TileContext(nc) as tc:
        with ExitStack() as outer:
            pools = (
                outer.enter_context(tc.tile_pool(name="const", bufs=1)),
                outer.enter_context(tc.tile_pool(name="dat", bufs=1)),
                outer.enter_context(tc.tile_pool(name="psS", bufs=2, space="PSUM")),
                outer.enter_context(tc.tile_pool(name="psB", bufs=2, space="PSUM")),
                outer.enter_context(tc.tile_pool(name="psO", bufs=2, space="PSUM")),
                outer.enter_context(tc.tile_pool(name="pAO", bufs=2)),
                outer.enter_context(tc.tile_pool(name="pE", bufs=6)),
                outer.enter_context(tc.tile_pool(name="pS", bufs=2)),
                outer.enter_context(tc.tile_pool(name="pOS", bufs=3)),
                outer.enter_context(tc.tile_pool(name="pQ", bufs=2)),
            )
            if loop_r is not None:
                with tc.For_i(0, loop_r, 1):
                    _emit_body(tc, pools)
            else:
                _emit_body(tc, pools)

    nc.compile()
    return nc


def _get_nc():
    if "nc" not in _STATE:
        _STATE["nc"] = _build()
    return _STATE["nc"]


def _make_in_maps(q, kv, Wq, Wk, Wv, Wo, bo):
    f16 = np.float16
    wq = np.asarray(Wq, dtype=np.float32).astype(f16)
    wk = np.asarray(Wk, dtype=np.float32).astype(f16)
    wv = np.asarray(Wv, dtype=np.float32).astype(f16)
    wo = np.asarray(Wo, dtype=np.float32).astype(f16)
    bo_bc = np.broadcast_to(
        np.asarray(bo, dtype=np.float32).reshape(1, DQ), (P, DQ)).copy()
    kvT = [np.ascontiguousarray(kv[b].T).astype(f16) for b in range(B)]
    in_maps = []
    for c in range(N_CORES):
        b, half = divmod(c, N_CORES // B)
        qT = np.ascontiguousarray(
            q[b, half * ROWS:(half + 1) * ROWS, :].T).astype(f16)
        in_maps.append({
            "qT": qT, "kvT": kvT[b],
            "wq": wq, "wk": wk, "wv": wv, "wo": wo, "bo_bc": bo_bc,
        })
    return in_maps


def kernel(q, kv, Wq, Wk, Wv, Wo, bo):
    from concourse.bass_utils import run_bass_kernel_spmd

    nc = _get_nc()
    in_maps = _make_in_maps(q, kv, Wq, Wk, Wv, Wo, bo)
    res = run_bass_kernel_spmd(nc, in_maps, list(range(N_CORES)))
    out = np.empty((B, LQ, DQ), dtype=np.float32)
    for c in range(N_CORES):
        b, half = divmod(c, N_CORES // B)
        out[b, half * ROWS:(half + 1) * ROWS, :] = res.results[c]["out"]
    return out


# revision 14
# speedup vs baseline: 1.0391x; 1.0391x over previous
"""Cross-attention Trainium2 kernel (8-core SPMD, no collectives).

Sharding: B*Lq = 4*2048 = 8192 query rows split 8 ways -> 1024 rows/core
(core c owns batch c//2, row-half c%2). Weights replicated. Each core:
  kp/vp projections of its batch's kv, qp projection of its q rows,
  per-head attention (softmax without max-subtraction; scores are O(1)
  by construction), output projection with the bias added during the
  PSUM->SBUF copy on the vector engine.

All matmul operands are fp16 (same 1 cycle/row PE rate as fp32r but
half the HBM/SBUF footprint); PSUM accumulation is fp32. Inputs are
converted to fp16 host-side. All tiles live at top level (no pool
scoping) so consecutive For_i iterations pipeline: the next
iteration's input DMAs (SP queue) overlap this iteration's compute,
while output DMAs drain on the separate ACT queue.
"""

import sys

import numpy as np

for _p in ("/opt/trn_rl_repo",):
    if _p not in sys.path:
        sys.path.insert(0, _p)

B, LQ, LKV = 4, 2048, 512
DQ, DKV = 1024, 768
H, HD = 8, 128
N_CORES = 8
ROWS = (B * LQ) // N_CORES  # 1024 q rows per core
P = 128
KTQ = DQ // P    # 8 k-tiles for dim_q contraction
KTK = DKV // P   # 6 k-tiles for dim_kv contraction
MC = ROWS // 512  # 2 m-chunks of 512 query rows
NT = LKV // P    # 4 kv n-tiles
SCALE = float(HD) ** -0.5

_STATE = {}


def _build(loop_r=None, phases="abc"):
    import concourse.bass as bass
    import concourse.mybir as mybir
    import concourse.tile as tile
    from concourse import bacc
    from concourse.bass import ts
    from contextlib import ExitStack

    f32 = mybir.dt.float32
    f16 = mybir.dt.float16
    Exp = mybir.ActivationFunctionType.Exp

    nc = bacc.Bacc("TRN2", target_bir_lowering=False, debug=False)
    qT_ap = nc.dram_tensor("qT", [DQ, ROWS], f16, kind="ExternalInput").ap()
    kvT_ap = nc.dram_tensor("kvT", [DKV, LKV], f16, kind="ExternalInput").ap()
    wq_ap = nc.dram_tensor("wq", [DQ, DQ], f16, kind="ExternalInput").ap()
    wk_ap = nc.dram_tensor("wk", [DKV, DQ], f16, kind="ExternalInput").ap()
    wv_ap = nc.dram_tensor("wv", [DKV, DQ], f16, kind="ExternalInput").ap()
    wo_ap = nc.dram_tensor("wo", [DQ, DQ], f16, kind="ExternalInput").ap()
    bo_ap = nc.dram_tensor("bo_bc", [P, DQ], f32, kind="ExternalInput").ap()
    out_ap = nc.dram_tensor("out", [ROWS, DQ], f32, kind="ExternalOutput").ap()

    qT_r = qT_ap.rearrange("(kt p) m -> p kt m", p=P)     # [128, 8, 1024]
    kvT_r = kvT_ap.rearrange("(kt p) n -> p kt n", p=P)   # [128, 6, 512]
    wq_r = wq_ap.rearrange("(kt p) n -> p kt n", p=P)     # [128, 8, 1024]
    wk_r = wk_ap.rearrange("(kt p) n -> p kt n", p=P)     # [128, 6, 1024]
    wv_r = wv_ap.rearrange("(kt p) n -> p kt n", p=P)     # [128, 6, 1024]
    wo_r = wo_ap.rearrange("(kt p) n -> p kt n", p=P)     # [128, 8, 1024]

    def _emit_body(tc, pl):
        const, dat, psS, psB, psO, psF, pAO, pE, pS, pOS, pQ = pl

        ones_col = const.tile([P, 1], f16, tag="ones_col")
        ones_row = const.tile([1, P], f16, tag="ones_row")
        bias2_sb = const.tile([P, 2, 512], f32, tag="bias")

        kvT_sb = dat.tile([P, KTK, LKV], f16, tag="kvT")
        wk_sb = dat.tile([P, KTK, DQ], f16, tag="wk")
        wv_sb = dat.tile([P, KTK, DQ], f16, tag="wv")
        wq_sb = dat.tile([P, KTQ, DQ], f16, tag="wq")
        qt_sb = dat.tile([P, KTQ, ROWS], f16, tag="qt")
        wo_sb = dat.tile([P, KTQ, DQ], f16, tag="wo")
        kpT = dat.tile([P, H, LKV], f16, tag="kpT")      # [d-in-head, h, kv]
        vp = dat.tile([P, NT, DQ], f16, tag="vp")        # [kv-in-tile, t, d]
        qpT0 = dat.tile([P, H, 512], f16, tag="qpT0")    # [d-in-head, h, m]
        qpT1 = dat.tile([P, H, 512], f16, tag="qpT1")
        qpTs = [qpT0, qpT1]

        # ones constants (memset writes f32; convert to f16 via DVE)
        ones_f = pS.tile([P, 1], f32, tag="onesf")
        nc.vector.memset(ones_f, 1.0)
        nc.vector.tensor_copy(ones_col, ones_f)
        onesrow_f = pS.tile([1, P], f32, tag="onesrf")
        nc.vector.memset(onesrow_f, 1.0)
        nc.vector.tensor_copy(ones_row, onesrow_f)

        # ---- Input DMAs (SP HWDGE queue), critical-path first:
        # interleave wk/kvT per-kt so the first kp matmul unblocks early.
        for kt in range(KTK):
            nc.sync.dma_start(out=wk_sb[:, kt, :], in_=wk_r[:, kt, :])
            nc.sync.dma_start(out=kvT_sb[:, kt, :], in_=kvT_r[:, kt, :])
        nc.sync.dma_start(out=wv_sb[:, 0:3, :], in_=wv_r[:, 0:3, :])
        nc.sync.dma_start(out=wv_sb[:, 3:6, :], in_=wv_r[:, 3:6, :])
        nc.sync.dma_start(out=wq_sb[:, 0:4, :], in_=wq_r[:, 0:4, :])
        nc.sync.dma_start(out=wq_sb[:, 4:8, :], in_=wq_r[:, 4:8, :])
        for mch in range(MC):
            nc.sync.dma_start(out=qt_sb[:, :, ts(mch, 512)],
                              in_=qT_r[:, :, ts(mch, 512)])
        nc.sync.dma_start(out=bias2_sb, in_=bo_ap)
        nc.sync.dma_start(out=wo_sb, in_=wo_r)

        # ---- Phase A: k/v projections
        for h in range(H):
            ps = psS.tile([P, 512], f32, tag="s")
            for kt in range(KTK):
                nc.tensor.matmul(ps, wk_sb[:, kt, ts(h, HD)],
                                 kvT_sb[:, kt, :],
                                 start=(kt == 0), stop=(kt == KTK - 1))
            nc.vector.tensor_copy(kpT[:, h, :], ps)
        for t in range(NT):
            for dc in range(2):
                ps = psS.tile([P, 512], f32, tag="s")
                for kt in range(KTK):
                    nc.tensor.matmul(ps, kvT_sb[:, kt, ts(t, P)],
                                     wv_sb[:, kt, ts(dc, 512)],
                                     start=(kt == 0), stop=(kt == KTK - 1))
                nc.vector.tensor_copy(vp[:, t, ts(dc, 512)], ps)

        # ---- Phase B: q projection -> qpT (N=512 matmuls)
        for mch in (range(MC) if "b" in phases else []):
            for h in range(H):
                ps = psS.tile([P, 512], f32, tag="s")
                for kt in range(KTQ):
                    nc.tensor.matmul(ps, wq_sb[:, kt, ts(h, HD)],
                                     qt_sb[:, kt, ts(mch, 512)],
                                     start=(kt == 0), stop=(kt == KTQ - 1))
                nc.vector.tensor_copy(qpTs[mch][:, h, :], ps)

        if "e" in phases:
            # probe: scores+exp pipeline only (no av/dn/out-proj).
            # Each exp is consumed by a DVE accumulate so nothing is DCE'd.
            acc0 = pS.tile([P, 512], f16, tag="eacc0", name="acc0")
            acc1 = pS.tile([P, 512], f16, tag="eacc1", name="acc1")
            acc = [acc0, acc1]
            nc.vector.memset(acc[0].bitcast(mybir.dt.uint16), 0)
            nc.vector.memset(acc[1].bitcast(mybir.dt.uint16), 0)
            i = 0
            for mc in range(MC):
                for h in range(H):
                    for t in range(NT):
                        ps_s = psS.tile([P, 512], f32, tag="s")
                        nc.tensor.matmul(ps_s, kpT[:, h, ts(t, P)],
                                         qpTs[mc][:, h, :],
                                         start=True, stop=True)
                        expT = pE.tile([P, 512], f16, tag="exp")
                        nc.scalar.activation(expT, ps_s, Exp, scale=SCALE)
                        nc.vector.tensor_add(acc[i % 2], acc[(i + 1) % 2], expT)
                        i += 1
            nc.scalar.dma_start(out=out_ap[4 * P:5 * P, 0:256].bitcast(f16),
                                in_=acc[(i + 1) % 2])
            return

        if "c" not in phases:
            # keep phase outputs live (avoid DCE) via sampled stores
            nc.scalar.dma_start(out=out_ap[0:P, 0:LKV // 2].bitcast(f16),
                                in_=kpT[:, 0, :])
            nc.scalar.dma_start(out=out_ap[P:2 * P, 0:DQ // 2].bitcast(f16),
                                in_=vp[:, 0, :])
            if "b" in phases:
                nc.scalar.dma_start(out=out_ap[2 * P:3 * P, 0:256].bitcast(f16),
                                    in_=qpT0[:, 0, :])
                nc.scalar.dma_start(out=out_ap[3 * P:4 * P, 0:256].bitcast(f16),
                                    in_=qpT1[:, 0, :])
            return

        # ---- Phase C: attention + output projection.
        # Denominator = DVE quadsum of the 4 exp tiles + one matmul into
        # row 0 of the (2-buf) broadcast PSUM tile; the broadcast matmul is
        # deferred to the NEXT head's t==1 slot so the reciprocal has ~850ns
        # of PE slack. Out DMAs ride the Pool SWDGE queue.
        def emit_tail(pending, aoT_of):
            ph, po, slot, prr = pending
            nc.tensor.matmul(slot, ones_row, prr, start=True, stop=True)
            bcast = pS.tile([P, 512], f32, tag="bc")
            nc.vector.tensor_copy(bcast, slot)
            nc.vector.tensor_mul(aoT_of[:, ph, :], po, bcast)

        for mc in range(MC):
            aoT = pAO.tile([P, H, 512], f16, tag="ao")
            pending = None  # (h, ps_o, psB slot, recip) tail of previous head
            for h in range(H):
                ps_o = psO.tile([P, 512], f32, tag="o")
                exps = []
                for t in range(NT):
                    ps_s = psS.tile([P, 512], f32, tag="s")
                    nc.tensor.matmul(ps_s, kpT[:, h, ts(t, P)],
                                     qpTs[mc][:, h, :],
                                     start=True, stop=True)
                    expT = pE.tile([P, 512], f16, tag="exp")
                    nc.scalar.activation(expT, ps_s, Exp, scale=SCALE)
                    exps.append(expT)
                    if t == 1 and pending is not None:
                        emit_tail(pending, aoT)
                        pending = None
                    nc.tensor.matmul(ps_o, vp[:, t, ts(h, HD)],
                                     expT,
                                     start=(t == 0), stop=(t == NT - 1))
                # denominator: sum the 4 exp tiles on DVE, single dn matmul
                s01 = pQ.tile([P, 512], f16, tag="s01")
                nc.vector.tensor_add(s01, exps[0], exps[1])
                s23 = pQ.tile([P, 512], f16, tag="s23")
                nc.vector.tensor_add(s23, exps[2], exps[3])
                ssum = pQ.tile([P, 512], f16, tag="ssum")
                nc.vector.tensor_add(ssum, s01, s23)
                slot = psB.tile([P, 512], f32, tag="b")
                nc.tensor.matmul(slot[0:1, :], ones_col, ssum,
                                 start=True, stop=True)
                recip_r = pS.tile([1, 512], f16, tag="rcr")
                with nc.allow_low_precision(reason="fp16 recip"):
                    nc.vector.reciprocal(recip_r, slot[0:1, :])
                pending = (h, ps_o, slot, recip_r)
            emit_tail(pending, aoT)
            # output projection for this m-chunk; bias added on DVE
            for mt in range(4):
                for nci in range(2):
                    ps_f = psF.tile([P, 512], f32, tag="f")
                    for kt in range(KTQ):
                        nc.tensor.matmul(ps_f, aoT[:, kt, ts(mt, P)],
                                         wo_sb[:, kt, ts(nci, 512)],
                                         start=(kt == 0), stop=(kt == KTQ - 1))
                    out_sb = pOS.tile([P, 512], f32, tag="osb")
                    nc.vector.tensor_add(out_sb, ps_f,
                                         bias2_sb[:, nci, :])
                    nc.gpsimd.dma_start(
                        out=out_ap[mc * 512 + mt * P: mc * 512 + (mt + 1) * P,
                                   ts(nci, 512)],
                        in_=out_sb)

    with tile.TileContext(nc) as tc:
        with ExitStack() as outer:
            pools = (
                outer.enter_context(tc.tile_pool(name="const", bufs=1)),
                outer.enter_context(tc.tile_pool(name="dat", bufs=1)),
                outer.enter_context(tc.tile_pool(name="psS", bufs=2, space="PSUM")),
                outer.enter_context(tc.tile_pool(name="psB", bufs=2, space="PSUM")),
                outer.enter_context(tc.tile_pool(name="psO", bufs=2, space="PSUM")),
                outer.enter_context(tc.tile_pool(name="psF", bufs=2, space="PSUM")),
                outer.enter_context(tc.tile_pool(name="pAO", bufs=2)),
                outer.enter_context(tc.tile_pool(name="pE", bufs=6)),
                outer.enter_context(tc.tile_pool(name="pS", bufs=2)),
                outer.enter_context(tc.tile_pool(name="pOS", bufs=3)),
                outer.enter_context(tc.tile_pool(name="pQ", bufs=2)),
            )
            if loop_r is not None:
                with tc.For_i(0, loop_r, 1):
                    _emit_body(tc, pools)
            else:
                _emit_body(tc, pools)

    nc.compile()
    return nc


def _get_nc():
    if "nc" not in _STATE:
        _STATE["nc"] = _build()
    return _STATE["nc"]


def _make_in_maps(q, kv, Wq, Wk, Wv, Wo, bo):
    f16 = np.float16
    wq = np.asarray(Wq, dtype=np.float32).astype(f16)
    wk = np.asarray(Wk, dtype=np.float32).astype(f16)
    wv = np.asarray(Wv, dtype=np.float32).astype(f16)
    wo = np.asarray(Wo, dtype=np.float32).astype(f16)
    bo_bc = np.broadcast_to(
        np.asarray(bo, dtype=np.float32).reshape(1, DQ), (P, DQ)).copy()
    kvT = [np.ascontiguousarray(kv[b].T).astype(f16) for b in range(B)]
    in_maps = []
    for c in range(N_CORES):
        b, half = divmod(c, N_CORES // B)
        qT = np.ascontiguousarray(
            q[b, half * ROWS:(half + 1) * ROWS, :].T).astype(f16)
        in_maps.append({
            "qT": qT, "kvT": kvT[b],
            "wq": wq, "wk": wk, "wv": wv, "wo": wo, "bo_bc": bo_bc,
        })
    return in_maps


def kernel(q, kv, Wq, Wk, Wv, Wo, bo):
    from concourse.bass_utils import run_bass_kernel_spmd

    nc = _get_nc()
    in_maps = _make_in_maps(q, kv, Wq, Wk, Wv, Wo, bo)
    res = run_bass_kernel_spmd(nc, in_maps, list(range(N_CORES)))
    out = np.empty((B, LQ, DQ), dtype=np.float32)
    for c in range(N_CORES):
        b, half = divmod(c, N_CORES // B)
        out[b, half * ROWS:(half + 1) * ROWS, :] = res.results[c]["out"]
    return out
